# revision 17
# baseline (speedup 1.0000x reference)
# Trainium2 Bass kernel for nn_LogitsNew (dense_mlp).
#
#   u = gelu(x @ W_proj + b_proj)                       [B, D]
#   logits = (u @ W_u)[:, None, :] + ee @ W_e           [B, N, C]
#
# Sharding: data-parallel over batch B across 8 cores (4 batches/core).
#
# fp16 end-to-end (tolerance 2e-2; fp16 keeps rel err ~4e-4), host-side
# layout transforms (k-chunk layouts, no PE transposes for the main path),
# fp16 stores (upcast on host). ~10MB HBM traffic per core; the two HWDGE
# rings process transfers serially with ~0.6us fixed cost each, so inputs
# ride in FEW, LARGE transfers (0.5-2MB), ordered by consumption time.
#
# Per core (PE order): mt0..mt2 | z,uT | mt3 | y,ybc | epilogue(mt0-3) |
# mt4..7 (sel-fused, drained by scalar+vector copies, stored immediately).
#   - main m-tile: accumulate eeT.T @ W_e into two PSUM banks (8 k each).
#   - z is transposed on the PE into ONE psum tile (8x [4,128] transposes
#     at disjoint column offsets), one Gelu produces uT [128, 8*4] directly.
#   - late m-tiles: a selector matmul (lhsT = e_b x ones_128, rhs =
#     y[4, 512] fp16) is appended to the PSUM group, so PSUM holds the
#     final logits.  Early m-tiles (drained to f32 before y exists) get
#     ybc via 4 PE broadcast-matmuls + DVE adds in a hoisted epilogue that
#     runs while the PE is on mt4..7.

import sys

if "/opt/trn_rl_repo" not in sys.path:
    sys.path.insert(0, "/opt/trn_rl_repo")

import numpy as np

import concourse.bass as bass
import concourse.mybir as mybir
import concourse.tile as tile
from concourse import bacc
from concourse.bass_utils import run_bass_kernel_spmd
from concourse.masks import make_identity

P = 128
B, N, D, C = 32, 256, 1024, 1024
NCORES = 8
BPC = B // NCORES          # batches per core
KT = D // P                # 8 k-tiles over the contraction dim
FD = 512                   # matmul moving free dim (one PSUM bank of fp32)
NT = N // P                # 2 n-tiles per batch
MT = BPC * NT              # 8 m-tiles per core
NEARLY = 3                 # m-tiles drained before y exists (epilogue add)
NBCAST = (NEARLY + NT - 1) // NT   # batches needing a broadcast y

F32 = mybir.dt.float32
F16 = mybir.dt.float16
GELU = mybir.ActivationFunctionType.Gelu

_CACHE = {}


def _build():
    if "nc" in _CACHE:
        return _CACHE["nc"]

    nc = bacc.Bacc("TRN2", target_bir_lowering=False, debug=False, num_devices=NCORES)

    # host-transformed inputs (fp16, k-chunk layouts, eet partition-major)
    eet = nc.dram_tensor("eet", [P, BPC, KT, N], F16, kind="ExternalInput").ap()
    we = nc.dram_tensor("we", [P, KT, C], F16, kind="ExternalInput").ap()
    wu = nc.dram_tensor("wu", [P, KT, C], F16, kind="ExternalInput").ap()
    wp = nc.dram_tensor("wp", [P, KT, C], F16, kind="ExternalInput").ap()
    xt = nc.dram_tensor("xt", [P, KT, BPC], F16, kind="ExternalInput").ap()
    bp = nc.dram_tensor("bp", [1, D], F16, kind="ExternalInput").ap()
    seld = nc.dram_tensor("sel", [BPC, BPC * P], F16, kind="ExternalInput").ap()
    out = nc.dram_tensor("logits", [BPC, N, C], F16, kind="ExternalOutput").ap()

    with tile.TileContext(nc) as tc:
        with (
            tc.tile_pool(name="const", bufs=1) as cpool,
            tc.tile_pool(name="outs", bufs=1) as outpool,
            tc.tile_pool(name="ost", bufs=4) as ostpool,
            tc.tile_pool(name="tp_ps", bufs=1, space="PSUM") as tp_ps,
            tc.tile_pool(name="warm_ps", bufs=1, space="PSUM") as warm_ps,
            tc.tile_pool(name="mm_ps", bufs=6, space="PSUM") as mm_ps,
        ):
            # ---- sync ring: W_e (small piece first for latency, then the
            # rest), ee1, W_u ----
            wesb = []   # k01 [P,2,C], k2-7 [P,6,C]
            for j, (k0, nk) in enumerate([(0, 2), (2, 6)]):
                t = cpool.tile([P, nk, C], F16, name=f"we_{j}")
                nc.sync.dma_start(t, we[:, k0 : k0 + nk])
                wesb.append(t)

            def we_at(k):
                return wesb[0][:, k] if k < 2 else wesb[1][:, k - 2]

            ee1 = cpool.tile([P, KT, N], F16, name="ee_1")
            nc.sync.dma_start(ee1, eet[:, 1])
            wusb = cpool.tile([P, KT, C], F16)
            nc.sync.dma_start(wusb, wu)

            # ---- scalar ring: ee0, W_proj, x/b/sel, ee2+ee3 ----
            ee0 = cpool.tile([P, KT, N], F16, name="ee_0")
            nc.scalar.dma_start(ee0, eet[:, 0])
            wpsb = cpool.tile([P, KT, C], F16)
            nc.scalar.dma_start(wpsb, wp)
            xsb = cpool.tile([P, KT, BPC], F16)
            nc.scalar.dma_start(xsb, xt)
            bsb = cpool.tile([1, D], F16)
            nc.scalar.dma_start(bsb, bp)
            sel = cpool.tile([BPC, BPC * P], F16)
            nc.scalar.dma_start(sel, seld)
            ee23 = cpool.tile([P, 2, KT, N], F16)
            nc.scalar.dma_start(ee23, eet[:, 2:4])

            def ee_at(b, k):
                if b == 0:
                    return ee0[:, k]
                if b == 1:
                    return ee1[:, k]
                return ee23[:, b - 2, k]

            # ---- constants (copies on the vector engine: the scalar queue
            # starts with ~4us of DMA issues) ----
            ident_f = cpool.tile([P, P], F32)
            make_identity(nc, ident_f)
            ident = cpool.tile([P, P], F16)
            nc.vector.tensor_copy(ident, ident_f)
            ones_f = cpool.tile([1, P], F32)
            nc.gpsimd.memset(ones_f, 1.0)
            ones = cpool.tile([1, P], F16)
            nc.vector.tensor_copy(ones, ones_f)

            # ---- PE warm-up: the tensor engine clock ramps over ~3.5us of
            # continuous work (matmuls run 2-3x slow until then).  Burn the
            # DMA-fill window on dummy matmuls so real work runs at full
            # clock from the first tile. ----
            warm = warm_ps.tile([P, P], F32, tag="warm")
            for _ in range(20):
                nc.tensor.matmul(warm, ident, ident, start=True, stop=True)

            zsb = cpool.tile([BPC, C], F16)
            uT = cpool.tile([P, KT * BPC], F16)
            ysb = cpool.tile([BPC, C], F16)
            ybc = cpool.tile([P, NBCAST, C], F32)

            o32 = []

            def utter_zu():
                # z = x @ W_proj + b
                for h in range(2):
                    cs = slice(h * FD, (h + 1) * FD)
                    zp = mm_ps.tile([P, FD], F32, tag="mm", name=f"z_{h}")
                    for k in range(KT):
                        nc.tensor.matmul(
                            zp[:BPC], xsb[:, k, :], wpsb[:, k, cs],
                            start=(k == 0), stop=False,
                        )
                    nc.tensor.matmul(
                        zp[:BPC], ones[:1, :BPC], bsb[:1, cs],
                        start=False, stop=True,
                    )
                    nc.vector.tensor_copy(zsb[:, cs], zp[:BPC])
                # transpose z into one psum tile, single Gelu -> uT
                tp = tp_ps.tile([P, KT * BPC], F16, tag="tp")
                for k in range(KT):
                    nc.tensor.transpose(
                        tp[:, k * BPC : (k + 1) * BPC],
                        zsb[:BPC, k * P : (k + 1) * P],
                        ident[:BPC, :BPC],
                    )
                nc.scalar.activation(uT, tp, GELU)

            def utter_y():
                # y = u @ W_u -> fp16 in partitions 0..3
                for h in range(2):
                    cs = slice(h * FD, (h + 1) * FD)
                    yp = mm_ps.tile([P, FD], F32, tag="mm", name=f"y_{h}")
                    for k in range(KT):
                        nc.tensor.matmul(
                            yp[:BPC], uT[:, k * BPC : (k + 1) * BPC],
                            wusb[:, k, cs],
                            start=(k == 0), stop=(k == KT - 1),
                        )
                    nc.vector.tensor_copy(ysb[:, cs], yp[:BPC])
                # ybc[:, b, :] = y[b] broadcast, for the early tiles' epilogue
                for b2 in range(NBCAST):
                    for ch in range(2):
                        cs = slice(ch * FD, (ch + 1) * FD)
                        bp_ = mm_ps.tile([P, FD], F32, tag="mm", name=f"yb{b2}{ch}")
                        nc.tensor.matmul(
                            bp_, sel[:, b2 * P : (b2 + 1) * P], ysb[:BPC, cs],
                            start=True, stop=True,
                        )
                        if ch == 0:
                            nc.scalar.copy(ybc[:, b2, cs], bp_)
                        else:
                            nc.vector.tensor_copy(ybc[:, b2, cs], bp_)

            def store(mt, o):
                b, nh = divmod(mt, NT)
                ns = slice(nh * P, (nh + 1) * P)
                eng = nc.sync if mt % 2 == 0 else nc.scalar
                eng.dma_start(out[b, ns, :], o.rearrange("p a f -> p (a f)"))

            for mt in range(MT):
                if mt == 2:
                    utter_zu()
                if mt == 3:
                    utter_y()
                    # hoisted epilogue: add y to the early tiles on the DVE
                    # and store them, while the PE works on mt4..7
                    for emt in range(NEARLY):
                        eb = emt // NT
                        o = ostpool.tile([P, 2, FD], F16, tag="ost", name=f"oste{emt}")
                        nc.vector.tensor_add(
                            o[:, 0, :], o32[emt][:, 0, :], ybc[:, eb, 0:FD]
                        )
                        nc.vector.tensor_add(
                            o[:, 1, :], o32[emt][:, 1, :], ybc[:, eb, FD:C]
                        )
                        store(emt, o)

                b, nh = divmod(mt, NT)
                ns = slice(nh * P, (nh + 1) * P)
                fuse_y = mt >= NEARLY
                mps = [
                    mm_ps.tile([P, FD], F32, tag="mm", name=f"mm_{mt}_{ch}")
                    for ch in range(2)
                ]
                for ch in range(2):
                    cs = slice(ch * FD, (ch + 1) * FD)
                    for k in range(KT):
                        nc.tensor.matmul(
                            mps[ch],
                            ee_at(b, k)[:, ns],
                            we_at(k)[:, cs],
                            start=(k == 0),
                            stop=(False if fuse_y else k == KT - 1),
                        )
                    if fuse_y:
                        # fuse the y broadcast-add into the accumulation
                        nc.tensor.matmul(
                            mps[ch], sel[:, b * P : (b + 1) * P], ysb[:BPC, cs],
                            start=False, stop=True,
                        )
                if mt < NEARLY:
                    # y not ready: drain to f32, add y in the hoisted epilogue
                    o = outpool.tile([P, 2, FD], F32, tag=f"o{mt}")
                    nc.scalar.copy(o[:, 0, :], mps[0])
                    nc.vector.tensor_copy(o[:, 1, :], mps[1])
                    o32.append(o)
                else:
                    # PSUM holds the final logits: drain fp16 on both engines
                    o = ostpool.tile([P, 2, FD], F16, tag="ost", name=f"ost{mt}")
                    nc.scalar.copy(o[:, 0, :], mps[0])
                    nc.vector.tensor_copy(o[:, 1, :], mps[1])
                    store(mt, o)

    nc.compile()
    _CACHE["nc"] = nc
    return nc


def _prep(inputs):
    """Host-side cast to fp16 + k-chunk layout transforms."""
    x = np.asarray(inputs["encoded_utterance"], np.float32)
    ee = np.asarray(inputs["element_embeddings"], np.float32)
    w = np.asarray(inputs["weight_matrix"], np.float32)
    wp = np.asarray(inputs["W_proj"], np.float32)
    bp = np.asarray(inputs["b_proj"], np.float32).reshape(1, D)

    # eet[p, b, k, n] = ee[b, n, k*128+p]  (partition-major)
    eet = np.ascontiguousarray(
        ee.reshape(B, N, KT, P).transpose(3, 0, 2, 1)
    ).astype(np.float16)

    # we/wu/wp[p, k, c] = W[k*128+p, c]
    def kchunk(m):
        return np.ascontiguousarray(
            m.reshape(KT, P, C).transpose(1, 0, 2)
        ).astype(np.float16)

    we_h = kchunk(w[D:])
    wu_h = kchunk(w[:D])
    wp_h = kchunk(wp)
    bp_h = bp.astype(np.float16)
    # xt[p, k, b] = x[b, k*128+p], per-core slice of b
    xt_full = np.ascontiguousarray(
        x.reshape(B, KT, P).transpose(2, 1, 0)
    ).astype(np.float16)
    sel_h = np.kron(np.eye(BPC), np.ones((1, P))).astype(np.float16)
    return eet, we_h, wu_h, wp_h, bp_h, xt_full, sel_h


def run(inputs, trace=False, **kwargs):
    nc = _build()
    eet, we_h, wu_h, wp_h, bp_h, xt_full, sel_h = _prep(inputs)

    in_maps = []
    for i in range(NCORES):
        bs = slice(i * BPC, (i + 1) * BPC)
        in_maps.append(
            {
                "eet": np.ascontiguousarray(eet[:, bs]),
                "we": we_h,
                "wu": wu_h,
                "wp": wp_h,
                "xt": np.ascontiguousarray(xt_full[:, :, bs]),
                "bp": bp_h,
                "sel": sel_h,
            }
        )

    res = run_bass_kernel_spmd(
        nc, in_maps, core_ids=list(range(NCORES)), trace=trace, **kwargs
    )
    full = np.concatenate([r["logits"] for r in res.results], axis=0)
    return full.astype(np.float32), res


def kernel(**inputs) -> np.ndarray:
    return run(inputs, trace=False)[0]


# revision 23
# speedup vs baseline: 1.0365x; 1.0365x over previous
# Trainium2 Bass kernel for nn_LogitsNew (dense_mlp).
#
#   u = gelu(x @ W_proj + b_proj)                       [B, D]
#   logits = (u @ W_u)[:, None, :] + ee @ W_e           [B, N, C]
#
# Sharding: data-parallel over batch B across 8 cores (4 batches/core).
#
# fp16 end-to-end (tolerance 2e-2; fp16 keeps rel err ~4e-4), host-side
# layout transforms (k-chunk layouts, no PE transposes for the main path),
# fp16 stores (upcast on host). ~10MB HBM traffic per core; the two HWDGE
# rings process transfers serially with ~0.6us fixed cost each, so inputs
# ride in FEW, LARGE transfers (0.5-2MB), ordered by consumption time.
#
# Per core (PE order): mt0..mt2 | z,uT | mt3 | y,ybc | epilogue(mt0-3) |
# mt4..7 (sel-fused, drained by scalar+vector copies, stored immediately).
#   - main m-tile: accumulate eeT.T @ W_e into two PSUM banks (8 k each).
#   - z is transposed on the PE into ONE psum tile (8x [4,128] transposes
#     at disjoint column offsets), one Gelu produces uT [128, 8*4] directly.
#   - late m-tiles: a selector matmul (lhsT = e_b x ones_128, rhs =
#     y[4, 512] fp16) is appended to the PSUM group, so PSUM holds the
#     final logits.  Early m-tiles (drained to f32 before y exists) get
#     ybc via 4 PE broadcast-matmuls + DVE adds in a hoisted epilogue that
#     runs while the PE is on mt4..7.

import sys

if "/opt/trn_rl_repo" not in sys.path:
    sys.path.insert(0, "/opt/trn_rl_repo")

import numpy as np

import concourse.bass as bass
import concourse.mybir as mybir
import concourse.tile as tile
from concourse import bacc
from concourse.bass_utils import run_bass_kernel_spmd
from concourse.masks import make_identity

P = 128
B, N, D, C = 32, 256, 1024, 1024
NCORES = 8
BPC = B // NCORES          # batches per core
KT = D // P                # 8 k-tiles over the contraction dim
FD = 512                   # matmul moving free dim (one PSUM bank of fp32)
NT = N // P                # 2 n-tiles per batch
MT = BPC * NT              # 8 m-tiles per core
NEARLY = 4                 # m-tiles drained before y exists (epilogue add)
NBCAST = (NEARLY + NT - 1) // NT   # batches needing a broadcast y

F32 = mybir.dt.float32
F16 = mybir.dt.float16
GELU = mybir.ActivationFunctionType.Gelu

_CACHE = {}


def _build():
    if "nc" in _CACHE:
        return _CACHE["nc"]

    nc = bacc.Bacc("TRN2", target_bir_lowering=False, debug=False, num_devices=NCORES)

    # host-transformed inputs (fp16, k-chunk layouts, eet partition-major)
    eet = nc.dram_tensor("eet", [P, BPC, KT, N], F16, kind="ExternalInput").ap()
    we = nc.dram_tensor("we", [P, KT, C], F16, kind="ExternalInput").ap()
    wu = nc.dram_tensor("wu", [P, KT, C], F16, kind="ExternalInput").ap()
    wp = nc.dram_tensor("wp", [P, KT, C], F16, kind="ExternalInput").ap()
    xt = nc.dram_tensor("xt", [P, KT, BPC], F16, kind="ExternalInput").ap()
    bp = nc.dram_tensor("bp", [1, D], F16, kind="ExternalInput").ap()
    seld = nc.dram_tensor("sel", [BPC, BPC * P], F16, kind="ExternalInput").ap()
    out = nc.dram_tensor("logits", [BPC, N, C], F16, kind="ExternalOutput").ap()

    with tile.TileContext(nc) as tc:
        with (
            tc.tile_pool(name="const", bufs=1) as cpool,
            tc.tile_pool(name="outs", bufs=1) as outpool,
            tc.tile_pool(name="ost", bufs=4) as ostpool,
            tc.tile_pool(name="tp_ps", bufs=1, space="PSUM") as tp_ps,
            tc.tile_pool(name="warm_ps", bufs=1, space="PSUM") as warm_ps,
            tc.tile_pool(name="mm_ps", bufs=6, space="PSUM") as mm_ps,
        ):
            # ---- both rings packed by consumption deadline:
            # sync:   we01 | we2-7 | wu_k0-3 | ee23
            # scalar: ee0 | ee1 | x/b/sel | wp | wu_k4-7 ----
            wesb = []   # k01 [P,2,C], k2-7 [P,6,C]
            for j, (k0, nk) in enumerate([(0, 2), (2, 6)]):
                t = cpool.tile([P, nk, C], F16, name=f"we_{j}")
                nc.sync.dma_start(t, we[:, k0 : k0 + nk])
                wesb.append(t)

            def we_at(k):
                return wesb[0][:, k] if k < 2 else wesb[1][:, k - 2]

            ee0 = cpool.tile([P, KT, N], F16, name="ee_0")
            nc.scalar.dma_start(ee0, eet[:, 0])
            ee1 = cpool.tile([P, KT, N], F16, name="ee_1")
            nc.scalar.dma_start(ee1, eet[:, 1])
            xsb = cpool.tile([P, KT, BPC], F16)
            nc.scalar.dma_start(xsb, xt)
            bsb = cpool.tile([1, D], F16)
            nc.scalar.dma_start(bsb, bp)
            sel = cpool.tile([BPC, BPC * P], F16)
            nc.scalar.dma_start(sel, seld)

            wusb = []
            t = cpool.tile([P, 4, C], F16, name="wu_0")
            nc.sync.dma_start(t, wu[:, :4])
            wusb.append(t)
            wpsb = cpool.tile([P, KT, C], F16)
            nc.scalar.dma_start(wpsb, wp)
            t = cpool.tile([P, 4, C], F16, name="wu_1")
            nc.scalar.dma_start(t, wu[:, 4:])
            wusb.append(t)
            ee23 = cpool.tile([P, 2, KT, N], F16)
            nc.sync.dma_start(ee23, eet[:, 2:4])

            def ee_at(b, k):
                if b == 0:
                    return ee0[:, k]
                if b == 1:
                    return ee1[:, k]
                return ee23[:, b - 2, k]

            # ---- constants (copies on the vector engine: the scalar queue
            # starts with ~4us of DMA issues) ----
            ident_f = cpool.tile([P, P], F32)
            make_identity(nc, ident_f)
            ident = cpool.tile([P, P], F16)
            nc.vector.tensor_copy(ident, ident_f)
            ones_f = cpool.tile([1, P], F32)
            nc.gpsimd.memset(ones_f, 1.0)
            ones = cpool.tile([1, P], F16)
            nc.vector.tensor_copy(ones, ones_f)

            # ---- PE warm-up: the tensor engine clock ramps over ~3.5us of
            # continuous work (matmuls run 2-3x slow until then).  Burn the
            # DMA-fill window on dummy matmuls so real work runs at full
            # clock from the first tile. ----
            warm = warm_ps.tile([P, P], F32, tag="warm")
            for _ in range(55):
                nc.tensor.matmul(warm, ident, ident, start=True, stop=True)

            zsb = cpool.tile([BPC, C], F16)
            uT = cpool.tile([P, KT * BPC], F16)
            ysb = cpool.tile([BPC, C], F16)
            ybc = cpool.tile([P, NBCAST, C], F32)

            o32 = []

            def utter_zu():
                # z = x @ W_proj + b
                for h in range(2):
                    cs = slice(h * FD, (h + 1) * FD)
                    zp = mm_ps.tile([P, FD], F32, tag="mm", name=f"z_{h}")
                    for k in range(KT):
                        nc.tensor.matmul(
                            zp[:BPC], xsb[:, k, :], wpsb[:, k, cs],
                            start=(k == 0), stop=False,
                        )
                    nc.tensor.matmul(
                        zp[:BPC], ones[:1, :BPC], bsb[:1, cs],
                        start=False, stop=True,
                    )
                    nc.vector.tensor_copy(zsb[:, cs], zp[:BPC])
                # transpose z into one psum tile, single Gelu -> uT
                tp = tp_ps.tile([P, KT * BPC], F16, tag="tp")
                for k in range(KT):
                    nc.tensor.transpose(
                        tp[:, k * BPC : (k + 1) * BPC],
                        zsb[:BPC, k * P : (k + 1) * P],
                        ident[:BPC, :BPC],
                    )
                nc.scalar.activation(uT, tp, GELU)

            def utter_y():
                # y = u @ W_u -> fp16 in partitions 0..3
                for h in range(2):
                    cs = slice(h * FD, (h + 1) * FD)
                    yp = mm_ps.tile([P, FD], F32, tag="mm", name=f"y_{h}")
                    for k in range(KT):
                        nc.tensor.matmul(
                            yp[:BPC], uT[:, k * BPC : (k + 1) * BPC],
                            wusb[k // 4][:, k % 4, cs],
                            start=(k == 0), stop=(k == KT - 1),
                        )
                    nc.vector.tensor_copy(ysb[:, cs], yp[:BPC])
                # ybc[:, b, :] = y[b] broadcast, for the early tiles' epilogue
                for b2 in range(NBCAST):
                    for ch in range(2):
                        cs = slice(ch * FD, (ch + 1) * FD)
                        bp_ = mm_ps.tile([P, FD], F32, tag="mm", name=f"yb{b2}{ch}")
                        nc.tensor.matmul(
                            bp_, sel[:, b2 * P : (b2 + 1) * P], ysb[:BPC, cs],
                            start=True, stop=True,
                        )
                        if ch == 0:
                            nc.scalar.copy(ybc[:, b2, cs], bp_)
                        else:
                            nc.vector.tensor_copy(ybc[:, b2, cs], bp_)

            def store(mt, o):
                b, nh = divmod(mt, NT)
                ns = slice(nh * P, (nh + 1) * P)
                eng = nc.sync if mt % 2 == 0 else nc.scalar
                eng.dma_start(out[b, ns, :], o.rearrange("p a f -> p (a f)"))

            for mt in range(MT):
                if mt == 3:
                    utter_zu()
                if mt == 4:
                    utter_y()
                    # hoisted epilogue: add y to the early tiles on the DVE
                    # and store them, while the PE works on mt4..7
                    for emt in range(NEARLY):
                        eb = emt // NT
                        o = ostpool.tile([P, 2, FD], F16, tag="ost", name=f"oste{emt}")
                        nc.vector.tensor_add(
                            o[:, 0, :], o32[emt][:, 0, :], ybc[:, eb, 0:FD]
                        )
                        nc.vector.tensor_add(
                            o[:, 1, :], o32[emt][:, 1, :], ybc[:, eb, FD:C]
                        )
                        store(emt, o)

                b, nh = divmod(mt, NT)
                ns = slice(nh * P, (nh + 1) * P)
                fuse_y = mt >= NEARLY
                mps = [
                    mm_ps.tile([P, FD], F32, tag="mm", name=f"mm_{mt}_{ch}")
                    for ch in range(2)
                ]
                for ch in range(2):
                    cs = slice(ch * FD, (ch + 1) * FD)
                    for k in range(KT):
                        nc.tensor.matmul(
                            mps[ch],
                            ee_at(b, k)[:, ns],
                            we_at(k)[:, cs],
                            start=(k == 0),
                            stop=(False if fuse_y else k == KT - 1),
                        )
                    if fuse_y:
                        # fuse the y broadcast-add into the accumulation
                        nc.tensor.matmul(
                            mps[ch], sel[:, b * P : (b + 1) * P], ysb[:BPC, cs],
                            start=False, stop=True,
                        )
                if mt < NEARLY:
                    # y not ready: drain to f32, add y in the hoisted epilogue
                    o = outpool.tile([P, 2, FD], F32, tag=f"o{mt}")
                    nc.scalar.copy(o[:, 0, :], mps[0])
                    nc.vector.tensor_copy(o[:, 1, :], mps[1])
                    o32.append(o)
                else:
                    # PSUM holds the final logits: drain fp16 on both engines
                    o = ostpool.tile([P, 2, FD], F16, tag="ost", name=f"ost{mt}")
                    nc.scalar.copy(o[:, 0, :], mps[0])
                    nc.vector.tensor_copy(o[:, 1, :], mps[1])
                    store(mt, o)

    nc.compile()
    _CACHE["nc"] = nc
    return nc


def _prep(inputs):
    """Host-side cast to fp16 + k-chunk layout transforms."""
    x = np.asarray(inputs["encoded_utterance"], np.float32)
    ee = np.asarray(inputs["element_embeddings"], np.float32)
    w = np.asarray(inputs["weight_matrix"], np.float32)
    wp = np.asarray(inputs["W_proj"], np.float32)
    bp = np.asarray(inputs["b_proj"], np.float32).reshape(1, D)

    # eet[p, b, k, n] = ee[b, n, k*128+p]  (partition-major)
    eet = np.ascontiguousarray(
        ee.reshape(B, N, KT, P).transpose(3, 0, 2, 1)
    ).astype(np.float16)

    # we/wu/wp[p, k, c] = W[k*128+p, c]
    def kchunk(m):
        return np.ascontiguousarray(
            m.reshape(KT, P, C).transpose(1, 0, 2)
        ).astype(np.float16)

    we_h = kchunk(w[D:])
    wu_h = kchunk(w[:D])
    wp_h = kchunk(wp)
    bp_h = bp.astype(np.float16)
    # xt[p, k, b] = x[b, k*128+p], per-core slice of b
    xt_full = np.ascontiguousarray(
        x.reshape(B, KT, P).transpose(2, 1, 0)
    ).astype(np.float16)
    sel_h = np.kron(np.eye(BPC), np.ones((1, P))).astype(np.float16)
    return eet, we_h, wu_h, wp_h, bp_h, xt_full, sel_h


def run(inputs, trace=False, **kwargs):
    nc = _build()
    eet, we_h, wu_h, wp_h, bp_h, xt_full, sel_h = _prep(inputs)

    in_maps = []
    for i in range(NCORES):
        bs = slice(i * BPC, (i + 1) * BPC)
        in_maps.append(
            {
                "eet": np.ascontiguousarray(eet[:, bs]),
                "we": we_h,
                "wu": wu_h,
                "wp": wp_h,
                "xt": np.ascontiguousarray(xt_full[:, :, bs]),
                "bp": bp_h,
                "sel": sel_h,
            }
        )

    res = run_bass_kernel_spmd(
        nc, in_maps, core_ids=list(range(NCORES)), trace=trace, **kwargs
    )
    full = np.concatenate([r["logits"] for r in res.results], axis=0)
    return full.astype(np.float32), res


def kernel(**inputs) -> np.ndarray:
    return run(inputs, trace=False)[0]


# revision 26
# speedup vs baseline: 1.0709x; 1.0332x over previous
# Trainium2 Bass kernel for nn_LogitsNew (dense_mlp).
#
#   u = gelu(x @ W_proj + b_proj)                       [B, D]
#   logits = (u @ W_u)[:, None, :] + ee @ W_e           [B, N, C]
#
# Sharding: data-parallel over batch B across 8 cores (4 batches/core).
#
# fp16 end-to-end (tolerance 2e-2; fp16 keeps rel err ~4e-4), host-side
# layout transforms (k-chunk layouts, no PE transposes for the main path),
# fp16 stores (upcast on host). ~10MB HBM traffic per core; the two HWDGE
# rings process transfers serially with ~0.6us fixed cost each, so inputs
# ride in FEW, LARGE transfers (0.5-2MB), ordered by consumption time.
#
# Per core (PE order): mt0..mt2 | z,uT | mt3 | y,ybc | epilogue(mt0-3) |
# mt4..7 (sel-fused, drained by scalar+vector copies, stored immediately).
#   - main m-tile: accumulate eeT.T @ W_e into two PSUM banks (8 k each).
#   - z is transposed on the PE into ONE psum tile (8x [4,128] transposes
#     at disjoint column offsets), one Gelu produces uT [128, 8*4] directly.
#   - late m-tiles: a selector matmul (lhsT = e_b x ones_128, rhs =
#     y[4, 512] fp16) is appended to the PSUM group, so PSUM holds the
#     final logits.  Early m-tiles (drained to f32 before y exists) get
#     ybc via 4 PE broadcast-matmuls + DVE adds in a hoisted epilogue that
#     runs while the PE is on mt4..7.

import sys

if "/opt/trn_rl_repo" not in sys.path:
    sys.path.insert(0, "/opt/trn_rl_repo")

import numpy as np

import concourse.bass as bass
import concourse.mybir as mybir
import concourse.tile as tile
from concourse import bacc
from concourse.bass_utils import run_bass_kernel_spmd
from concourse.masks import make_identity

P = 128
B, N, D, C = 32, 256, 1024, 1024
NCORES = 8
BPC = B // NCORES          # batches per core
KT = D // P                # 8 k-tiles over the contraction dim
FD = 512                   # matmul moving free dim (one PSUM bank of fp32)
NT = N // P                # 2 n-tiles per batch
MT = BPC * NT              # 8 m-tiles per core
NEARLY = 4                 # m-tiles drained before y exists (epilogue add)
NBCAST = (NEARLY + NT - 1) // NT   # batches needing a broadcast y

F32 = mybir.dt.float32
F16 = mybir.dt.float16
GELU = mybir.ActivationFunctionType.Gelu

_CACHE = {}


def _build():
    if "nc" in _CACHE:
        return _CACHE["nc"]

    nc = bacc.Bacc("TRN2", target_bir_lowering=False, debug=False, num_devices=NCORES)

    # host-transformed inputs (fp16, k-chunk layouts, eet partition-major)
    eet = nc.dram_tensor("eet", [P, BPC, KT, N], F16, kind="ExternalInput").ap()
    we = nc.dram_tensor("we", [P, KT, C], F16, kind="ExternalInput").ap()
    wu = nc.dram_tensor("wu", [P, KT, C], F16, kind="ExternalInput").ap()
    wp = nc.dram_tensor("wp", [P, KT, C], F16, kind="ExternalInput").ap()
    xt = nc.dram_tensor("xt", [P, KT, BPC], F16, kind="ExternalInput").ap()
    bp = nc.dram_tensor("bp", [1, D], F16, kind="ExternalInput").ap()
    seld = nc.dram_tensor("sel", [BPC, BPC * P], F16, kind="ExternalInput").ap()
    out = nc.dram_tensor("logits", [BPC, N, C], F16, kind="ExternalOutput").ap()

    with tile.TileContext(nc) as tc:
        with (
            tc.tile_pool(name="const", bufs=1) as cpool,
            tc.tile_pool(name="outs", bufs=1) as outpool,
            tc.tile_pool(name="ost", bufs=4) as ostpool,
            tc.tile_pool(name="tp_ps", bufs=1, space="PSUM") as tp_ps,
            tc.tile_pool(name="warm_ps", bufs=1, space="PSUM") as warm_ps,
            tc.tile_pool(name="mm_ps", bufs=6, space="PSUM") as mm_ps,
        ):
            # ---- both rings packed by consumption deadline (~175GB/s per
            # ring under contention):
            # sync:   we01|we23|we45|we67 ladder | wp_k4-7 | wu_k0-3
            # scalar: ee0 | ee1 | x/b/sel | wp_k0-3 | wu_k4-7 | ee23 ----
            wesb = []   # 4x [P,2,C]
            for j in range(4):
                t = cpool.tile([P, 2, C], F16, name=f"we_{j}")
                nc.sync.dma_start(t, we[:, 2 * j : 2 * j + 2])
                wesb.append(t)

            def we_at(k):
                return wesb[k // 2][:, k % 2]

            ee0 = cpool.tile([P, KT, N], F16, name="ee_0")
            nc.scalar.dma_start(ee0, eet[:, 0])
            ee1 = cpool.tile([P, KT, N], F16, name="ee_1")
            nc.scalar.dma_start(ee1, eet[:, 1])
            xsb = cpool.tile([P, KT, BPC], F16)
            nc.scalar.dma_start(xsb, xt)
            bsb = cpool.tile([1, D], F16)
            nc.scalar.dma_start(bsb, bp)
            sel = cpool.tile([BPC, BPC * P], F16)
            nc.scalar.dma_start(sel, seld)

            wpsb = []
            t = cpool.tile([P, 4, C], F16, name="wp_0")
            nc.scalar.dma_start(t, wp[:, :4])
            wpsb.append(t)
            t = cpool.tile([P, 4, C], F16, name="wp_1")
            nc.sync.dma_start(t, wp[:, 4:])
            wpsb.append(t)
            wusb = []
            t = cpool.tile([P, 4, C], F16, name="wu_0")
            nc.sync.dma_start(t, wu[:, :4])
            wusb.append(t)
            t = cpool.tile([P, 4, C], F16, name="wu_1")
            nc.scalar.dma_start(t, wu[:, 4:])
            wusb.append(t)
            ee23 = cpool.tile([P, 2, KT, N], F16)
            nc.scalar.dma_start(ee23, eet[:, 2:4])

            def ee_at(b, k):
                if b == 0:
                    return ee0[:, k]
                if b == 1:
                    return ee1[:, k]
                return ee23[:, b - 2, k]

            # ---- constants (copies on the vector engine: the scalar queue
            # starts with ~4us of DMA issues) ----
            ident_f = cpool.tile([P, P], F32)
            make_identity(nc, ident_f)
            ident = cpool.tile([P, P], F16)
            nc.vector.tensor_copy(ident, ident_f)
            ones_f = cpool.tile([1, P], F32)
            nc.gpsimd.memset(ones_f, 1.0)
            ones = cpool.tile([1, P], F16)
            nc.vector.tensor_copy(ones, ones_f)

            # ---- PE warm-up: the tensor engine clock ramps over ~3.5us of
            # continuous work (matmuls run 2-3x slow until then).  Burn the
            # DMA-fill window on dummy matmuls so real work runs at full
            # clock from the first tile. ----
            warm = warm_ps.tile([P, P], F32, tag="warm")
            for _ in range(55):
                nc.tensor.matmul(warm, ident, ident, start=True, stop=True)

            zsb = cpool.tile([BPC, C], F16)
            uT = cpool.tile([P, KT * BPC], F16)
            ysb = cpool.tile([BPC, C], F16)
            ybc = cpool.tile([P, NBCAST, C], F32)

            o32 = []

            def utter_zu():
                # z = x @ W_proj + b
                for h in range(2):
                    cs = slice(h * FD, (h + 1) * FD)
                    zp = mm_ps.tile([P, FD], F32, tag="mm", name=f"z_{h}")
                    for k in range(KT):
                        nc.tensor.matmul(
                            zp[:BPC], xsb[:, k, :], wpsb[k // 4][:, k % 4, cs],
                            start=(k == 0), stop=False,
                        )
                    nc.tensor.matmul(
                        zp[:BPC], ones[:1, :BPC], bsb[:1, cs],
                        start=False, stop=True,
                    )
                    nc.vector.tensor_copy(zsb[:, cs], zp[:BPC])
                # transpose z into one psum tile, single Gelu -> uT
                tp = tp_ps.tile([P, KT * BPC], F16, tag="tp")
                for k in range(KT):
                    nc.tensor.transpose(
                        tp[:, k * BPC : (k + 1) * BPC],
                        zsb[:BPC, k * P : (k + 1) * P],
                        ident[:BPC, :BPC],
                    )
                nc.scalar.activation(uT, tp, GELU)

            def utter_y():
                # y = u @ W_u -> fp16 in partitions 0..3
                for h in range(2):
                    cs = slice(h * FD, (h + 1) * FD)
                    yp = mm_ps.tile([P, FD], F32, tag="mm", name=f"y_{h}")
                    for k in range(KT):
                        nc.tensor.matmul(
                            yp[:BPC], uT[:, k * BPC : (k + 1) * BPC],
                            wusb[k // 4][:, k % 4, cs],
                            start=(k == 0), stop=(k == KT - 1),
                        )
                    nc.vector.tensor_copy(ysb[:, cs], yp[:BPC])
                # ybc[:, b, :] = y[b] broadcast, for the early tiles' epilogue
                for b2 in range(NBCAST):
                    for ch in range(2):
                        cs = slice(ch * FD, (ch + 1) * FD)
                        bp_ = mm_ps.tile([P, FD], F32, tag="mm", name=f"yb{b2}{ch}")
                        nc.tensor.matmul(
                            bp_, sel[:, b2 * P : (b2 + 1) * P], ysb[:BPC, cs],
                            start=True, stop=True,
                        )
                        if ch == 0:
                            nc.scalar.copy(ybc[:, b2, cs], bp_)
                        else:
                            nc.vector.tensor_copy(ybc[:, b2, cs], bp_)

            def store(mt, o):
                b, nh = divmod(mt, NT)
                ns = slice(nh * P, (nh + 1) * P)
                eng = nc.sync if mt % 2 == 0 else nc.scalar
                eng.dma_start(out[b, ns, :], o.rearrange("p a f -> p (a f)"))

            # ---- m-tiles 0+1 interleaved k-major: consumption paced to the
            # W_e arrival ladder; warm-fill matmuls plug the predicted
            # sub-us supply gaps so the PE clock never drops ----
            mpsA = [
                [
                    mm_ps.tile([P, FD], F32, tag="mm", name=f"mm_{mt}_{ch}")
                    for ch in range(2)
                ]
                for mt in range(2)
            ]
            WARMF = {4: 8, 6: 22}
            for k in range(KT):
                for _ in range(WARMF.get(k, 0)):
                    nc.tensor.matmul(warm, ident, ident, start=True, stop=True)
                for mt in range(2):
                    ns = slice(mt * P, (mt + 1) * P)
                    for ch in range(2):
                        cs = slice(ch * FD, (ch + 1) * FD)
                        nc.tensor.matmul(
                            mpsA[mt][ch],
                            ee0[:, k][:, ns],
                            we_at(k)[:, cs],
                            start=(k == 0),
                            stop=(k == KT - 1),
                        )
            for mt in range(2):
                o = outpool.tile([P, 2, FD], F32, tag=f"o{mt}")
                nc.scalar.copy(o[:, 0, :], mpsA[mt][0])
                nc.vector.tensor_copy(o[:, 1, :], mpsA[mt][1])
                o32.append(o)

            for mt in range(2, MT):
                if mt == 3:
                    utter_zu()
                if mt == 4:
                    utter_y()
                    # hoisted epilogue: add y to the early tiles on the DVE
                    # and store them, while the PE works on mt4..7
                    for emt in range(NEARLY):
                        eb = emt // NT
                        o = ostpool.tile([P, 2, FD], F16, tag="ost", name=f"oste{emt}")
                        nc.vector.tensor_add(
                            o[:, 0, :], o32[emt][:, 0, :], ybc[:, eb, 0:FD]
                        )
                        nc.vector.tensor_add(
                            o[:, 1, :], o32[emt][:, 1, :], ybc[:, eb, FD:C]
                        )
                        store(emt, o)

                b, nh = divmod(mt, NT)
                ns = slice(nh * P, (nh + 1) * P)
                fuse_y = mt >= NEARLY
                mps = [
                    mm_ps.tile([P, FD], F32, tag="mm", name=f"mm_{mt}_{ch}")
                    for ch in range(2)
                ]
                for ch in range(2):
                    cs = slice(ch * FD, (ch + 1) * FD)
                    for k in range(KT):
                        nc.tensor.matmul(
                            mps[ch],
                            ee_at(b, k)[:, ns],
                            we_at(k)[:, cs],
                            start=(k == 0),
                            stop=(False if fuse_y else k == KT - 1),
                        )
                    if fuse_y:
                        # fuse the y broadcast-add into the accumulation
                        nc.tensor.matmul(
                            mps[ch], sel[:, b * P : (b + 1) * P], ysb[:BPC, cs],
                            start=False, stop=True,
                        )
                if mt < NEARLY:
                    # y not ready: drain to f32, add y in the hoisted epilogue
                    o = outpool.tile([P, 2, FD], F32, tag=f"o{mt}")
                    nc.scalar.copy(o[:, 0, :], mps[0])
                    nc.vector.tensor_copy(o[:, 1, :], mps[1])
                    o32.append(o)
                else:
                    # PSUM holds the final logits: drain fp16 on both engines
                    o = ostpool.tile([P, 2, FD], F16, tag="ost", name=f"ost{mt}")
                    nc.scalar.copy(o[:, 0, :], mps[0])
                    nc.vector.tensor_copy(o[:, 1, :], mps[1])
                    store(mt, o)

    nc.compile()
    _CACHE["nc"] = nc
    return nc


def _prep(inputs):
    """Host-side cast to fp16 + k-chunk layout transforms."""
    x = np.asarray(inputs["encoded_utterance"], np.float32)
    ee = np.asarray(inputs["element_embeddings"], np.float32)
    w = np.asarray(inputs["weight_matrix"], np.float32)
    wp = np.asarray(inputs["W_proj"], np.float32)
    bp = np.asarray(inputs["b_proj"], np.float32).reshape(1, D)

    # eet[p, b, k, n] = ee[b, n, k*128+p]  (partition-major)
    eet = np.ascontiguousarray(
        ee.reshape(B, N, KT, P).transpose(3, 0, 2, 1)
    ).astype(np.float16)

    # we/wu/wp[p, k, c] = W[k*128+p, c]
    def kchunk(m):
        return np.ascontiguousarray(
            m.reshape(KT, P, C).transpose(1, 0, 2)
        ).astype(np.float16)

    we_h = kchunk(w[D:])
    wu_h = kchunk(w[:D])
    wp_h = kchunk(wp)
    bp_h = bp.astype(np.float16)
    # xt[p, k, b] = x[b, k*128+p], per-core slice of b
    xt_full = np.ascontiguousarray(
        x.reshape(B, KT, P).transpose(2, 1, 0)
    ).astype(np.float16)
    sel_h = np.kron(np.eye(BPC), np.ones((1, P))).astype(np.float16)
    return eet, we_h, wu_h, wp_h, bp_h, xt_full, sel_h


def run(inputs, trace=False, **kwargs):
    nc = _build()
    eet, we_h, wu_h, wp_h, bp_h, xt_full, sel_h = _prep(inputs)

    in_maps = []
    for i in range(NCORES):
        bs = slice(i * BPC, (i + 1) * BPC)
        in_maps.append(
            {
                "eet": np.ascontiguousarray(eet[:, bs]),
                "we": we_h,
                "wu": wu_h,
                "wp": wp_h,
                "xt": np.ascontiguousarray(xt_full[:, :, bs]),
                "bp": bp_h,
                "sel": sel_h,
            }
        )

    res = run_bass_kernel_spmd(
        nc, in_maps, core_ids=list(range(NCORES)), trace=trace, **kwargs
    )
    full = np.concatenate([r["logits"] for r in res.results], axis=0)
    return full.astype(np.float32), res


def kernel(**inputs) -> np.ndarray:
    return run(inputs, trace=False)[0]


# revision 29
# speedup vs baseline: 1.0793x; 1.0078x over previous
# Trainium2 Bass kernel for nn_LogitsNew (dense_mlp).
#
#   u = gelu(x @ W_proj + b_proj)                       [B, D]
#   logits = (u @ W_u)[:, None, :] + ee @ W_e           [B, N, C]
#
# Sharding: data-parallel over batch B across 8 cores (4 batches/core).
#
# fp16 end-to-end (tolerance 2e-2; fp16 keeps rel err ~4e-4), host-side
# layout transforms (k-chunk layouts, no PE transposes for the main path),
# fp16 stores (upcast on host). ~10MB HBM traffic per core; the two HWDGE
# rings process transfers serially with ~0.6us fixed cost each, so inputs
# ride in FEW, LARGE transfers (0.5-2MB), ordered by consumption time.
#
# Per core (PE order): mt0..mt2 | z,uT | mt3 | y,ybc | epilogue(mt0-3) |
# mt4..7 (sel-fused, drained by scalar+vector copies, stored immediately).
#   - main m-tile: accumulate eeT.T @ W_e into two PSUM banks (8 k each).
#   - z is transposed on the PE into ONE psum tile (8x [4,128] transposes
#     at disjoint column offsets), one Gelu produces uT [128, 8*4] directly.
#   - late m-tiles: a selector matmul (lhsT = e_b x ones_128, rhs =
#     y[4, 512] fp16) is appended to the PSUM group, so PSUM holds the
#     final logits.  Early m-tiles (drained to f32 before y exists) get
#     ybc via 4 PE broadcast-matmuls + DVE adds in a hoisted epilogue that
#     runs while the PE is on mt4..7.

import sys

if "/opt/trn_rl_repo" not in sys.path:
    sys.path.insert(0, "/opt/trn_rl_repo")

import numpy as np

import concourse.bass as bass
import concourse.mybir as mybir
import concourse.tile as tile
from concourse import bacc
from concourse.bass_utils import run_bass_kernel_spmd
from concourse.masks import make_identity

P = 128
B, N, D, C = 32, 256, 1024, 1024
NCORES = 8
BPC = B // NCORES          # batches per core
KT = D // P                # 8 k-tiles over the contraction dim
FD = 512                   # matmul moving free dim (one PSUM bank of fp32)
NT = N // P                # 2 n-tiles per batch
MT = BPC * NT              # 8 m-tiles per core
NEARLY = 4                 # m-tiles drained before y exists (epilogue add)
NBCAST = (NEARLY + NT - 1) // NT   # batches needing a broadcast y

F32 = mybir.dt.float32
F16 = mybir.dt.float16
GELU = mybir.ActivationFunctionType.Gelu

_CACHE = {}


def _build():
    if "nc" in _CACHE:
        return _CACHE["nc"]

    nc = bacc.Bacc("TRN2", target_bir_lowering=False, debug=False, num_devices=NCORES)

    # host-transformed inputs (fp16, k-chunk layouts, eet partition-major)
    eet = nc.dram_tensor("eet", [P, BPC, KT, N], F16, kind="ExternalInput").ap()
    we = nc.dram_tensor("we", [P, KT, C], F16, kind="ExternalInput").ap()
    wu = nc.dram_tensor("wu", [P, KT, C], F16, kind="ExternalInput").ap()
    wp = nc.dram_tensor("wp", [P, KT, C], F16, kind="ExternalInput").ap()
    xt = nc.dram_tensor("xt", [P, KT, BPC], F16, kind="ExternalInput").ap()
    bp = nc.dram_tensor("bp", [1, D], F16, kind="ExternalInput").ap()
    seld = nc.dram_tensor("sel", [BPC, BPC * P], F16, kind="ExternalInput").ap()
    out = nc.dram_tensor("logits", [BPC, N, C], F16, kind="ExternalOutput").ap()

    with tile.TileContext(nc) as tc:
        with (
            tc.tile_pool(name="const", bufs=1) as cpool,
            tc.tile_pool(name="outs", bufs=1) as outpool,
            tc.tile_pool(name="ost", bufs=4) as ostpool,
            tc.tile_pool(name="tp_ps", bufs=1, space="PSUM") as tp_ps,
            tc.tile_pool(name="warm_ps", bufs=1, space="PSUM") as warm_ps,
            tc.tile_pool(name="mm_ps", bufs=6, space="PSUM") as mm_ps,
        ):
            # ---- both rings packed by consumption deadline (~175GB/s per
            # ring under contention):
            # sync:   we01|we23|we45|we67 ladder | wp_k4-7 | wu_k0-3
            # scalar: ee0 | ee1 | x/b/sel | wp_k0-3 | wu_k4-7 | ee23 ----
            wesb = []   # 4x [P,2,C]
            for j in range(4):
                t = cpool.tile([P, 2, C], F16, name=f"we_{j}")
                nc.sync.dma_start(t, we[:, 2 * j : 2 * j + 2])
                wesb.append(t)

            def we_at(k):
                return wesb[k // 2][:, k % 2]

            ee0 = cpool.tile([P, KT, N], F16, name="ee_0")
            nc.scalar.dma_start(ee0, eet[:, 0])
            ee1 = cpool.tile([P, KT, N], F16, name="ee_1")
            nc.scalar.dma_start(ee1, eet[:, 1])
            xsb = cpool.tile([P, KT, BPC], F16)
            nc.scalar.dma_start(xsb, xt)
            bsb = cpool.tile([1, D], F16)
            nc.scalar.dma_start(bsb, bp)
            sel = cpool.tile([BPC, BPC * P], F16)
            nc.scalar.dma_start(sel, seld)

            wpsb = []
            t = cpool.tile([P, 4, C], F16, name="wp_0")
            nc.scalar.dma_start(t, wp[:, :4])
            wpsb.append(t)
            t = cpool.tile([P, 4, C], F16, name="wp_1")
            nc.sync.dma_start(t, wp[:, 4:])
            wpsb.append(t)
            wusb = []
            t = cpool.tile([P, 4, C], F16, name="wu_0")
            nc.sync.dma_start(t, wu[:, :4])
            wusb.append(t)
            t = cpool.tile([P, 4, C], F16, name="wu_1")
            nc.scalar.dma_start(t, wu[:, 4:])
            wusb.append(t)
            ee23 = cpool.tile([P, 2, KT, N], F16)
            nc.scalar.dma_start(ee23, eet[:, 2:4])

            def ee_at(b, k):
                if b == 0:
                    return ee0[:, k]
                if b == 1:
                    return ee1[:, k]
                return ee23[:, b - 2, k]

            # ---- constants (copies on the vector engine: the scalar queue
            # starts with ~4us of DMA issues) ----
            ident_f = cpool.tile([P, P], F32)
            make_identity(nc, ident_f)
            ident = cpool.tile([P, P], F16)
            nc.vector.tensor_copy(ident, ident_f)
            ones_f = cpool.tile([1, P], F32)
            nc.gpsimd.memset(ones_f, 1.0)
            ones = cpool.tile([1, P], F16)
            nc.vector.tensor_copy(ones, ones_f)

            # ---- PE warm-up: the tensor engine clock ramps over ~3.5us of
            # continuous work (matmuls run 2-3x slow until then).  Burn the
            # DMA-fill window on dummy matmuls so real work runs at full
            # clock from the first tile. ----
            warm = warm_ps.tile([P, P], F32, tag="warm")
            for _ in range(70):
                nc.tensor.matmul(warm, ident, ident, start=True, stop=True)

            zsb = cpool.tile([BPC, C], F16)
            uT = cpool.tile([P, KT * BPC], F16)
            ysb = cpool.tile([BPC, C], F16)
            ybc = cpool.tile([P, NBCAST, C], F32)

            o32 = []

            def utter_zu():
                # z = x @ W_proj + b
                for h in range(2):
                    cs = slice(h * FD, (h + 1) * FD)
                    zp = mm_ps.tile([P, FD], F32, tag="mm", name=f"z_{h}")
                    for k in range(KT):
                        nc.tensor.matmul(
                            zp[:BPC], xsb[:, k, :], wpsb[k // 4][:, k % 4, cs],
                            start=(k == 0), stop=False,
                        )
                    nc.tensor.matmul(
                        zp[:BPC], ones[:1, :BPC], bsb[:1, cs],
                        start=False, stop=True,
                    )
                    nc.vector.tensor_copy(zsb[:, cs], zp[:BPC])
                # transpose z into one psum tile, single Gelu -> uT
                tp = tp_ps.tile([P, KT * BPC], F16, tag="tp")
                for k in range(KT):
                    nc.tensor.transpose(
                        tp[:, k * BPC : (k + 1) * BPC],
                        zsb[:BPC, k * P : (k + 1) * P],
                        ident[:BPC, :BPC],
                    )
                nc.scalar.activation(uT, tp, GELU)

            def utter_y():
                # y = u @ W_u -> fp16 in partitions 0..3
                for h in range(2):
                    cs = slice(h * FD, (h + 1) * FD)
                    yp = mm_ps.tile([P, FD], F32, tag="mm", name=f"y_{h}")
                    for k in range(KT):
                        nc.tensor.matmul(
                            yp[:BPC], uT[:, k * BPC : (k + 1) * BPC],
                            wusb[k // 4][:, k % 4, cs],
                            start=(k == 0), stop=(k == KT - 1),
                        )
                    nc.vector.tensor_copy(ysb[:, cs], yp[:BPC])
                # ybc[:, b, :] = y[b] broadcast, for the early tiles' epilogue
                for b2 in range(NBCAST):
                    for ch in range(2):
                        cs = slice(ch * FD, (ch + 1) * FD)
                        bp_ = mm_ps.tile([P, FD], F32, tag="mm", name=f"yb{b2}{ch}")
                        nc.tensor.matmul(
                            bp_, sel[:, b2 * P : (b2 + 1) * P], ysb[:BPC, cs],
                            start=True, stop=True,
                        )
                        if ch == 0:
                            nc.scalar.copy(ybc[:, b2, cs], bp_)
                        else:
                            nc.vector.tensor_copy(ybc[:, b2, cs], bp_)

            def store(mt, o):
                b, nh = divmod(mt, NT)
                ns = slice(nh * P, (nh + 1) * P)
                if mt >= MT - 2:
                    # tail tiles: split the store across both rings so the
                    # last transfer is half-size
                    ov = out[b, ns, :].rearrange("p (a f) -> p a f", a=2)
                    nc.sync.dma_start(ov[:, 0], o[:, 0, :])
                    nc.scalar.dma_start(ov[:, 1], o[:, 1, :])
                else:
                    eng = nc.sync if mt % 2 == 0 else nc.scalar
                    eng.dma_start(out[b, ns, :], o.rearrange("p a f -> p (a f)"))

            # ---- m-tiles 0+1 interleaved k-major: consumption paced to the
            # W_e arrival ladder; warm-fill matmuls plug the predicted
            # sub-us supply gaps so the PE clock never drops ----
            mpsA = [
                [
                    mm_ps.tile([P, FD], F32, tag="mm", name=f"mm_{mt}_{ch}")
                    for ch in range(2)
                ]
                for mt in range(2)
            ]
            WARMF = {2: 6, 4: 8}
            for k in range(KT):
                for _ in range(WARMF.get(k, 0)):
                    nc.tensor.matmul(warm, ident, ident, start=True, stop=True)
                for mt in range(2):
                    ns = slice(mt * P, (mt + 1) * P)
                    for ch in range(2):
                        cs = slice(ch * FD, (ch + 1) * FD)
                        nc.tensor.matmul(
                            mpsA[mt][ch],
                            ee0[:, k][:, ns],
                            we_at(k)[:, cs],
                            start=(k == 0),
                            stop=(k == KT - 1),
                        )
            for mt in range(2):
                o = outpool.tile([P, 2, FD], F32, tag=f"o{mt}")
                nc.scalar.copy(o[:, 0, :], mpsA[mt][0])
                nc.vector.tensor_copy(o[:, 1, :], mpsA[mt][1])
                o32.append(o)

            for mt in range(2, MT):
                if mt == 3:
                    utter_zu()
                if mt == 4:
                    utter_y()
                    # hoisted epilogue: add y to the early tiles on the DVE
                    # and store them, while the PE works on mt4..7
                    for emt in range(NEARLY):
                        eb = emt // NT
                        o = ostpool.tile([P, 2, FD], F16, tag="ost", name=f"oste{emt}")
                        nc.vector.tensor_add(
                            o[:, 0, :], o32[emt][:, 0, :], ybc[:, eb, 0:FD]
                        )
                        nc.vector.tensor_add(
                            o[:, 1, :], o32[emt][:, 1, :], ybc[:, eb, FD:C]
                        )
                        store(emt, o)

                b, nh = divmod(mt, NT)
                ns = slice(nh * P, (nh + 1) * P)
                fuse_y = mt >= NEARLY
                mps = [
                    mm_ps.tile([P, FD], F32, tag="mm", name=f"mm_{mt}_{ch}")
                    for ch in range(2)
                ]
                for ch in range(2):
                    cs = slice(ch * FD, (ch + 1) * FD)
                    for k in range(KT):
                        nc.tensor.matmul(
                            mps[ch],
                            ee_at(b, k)[:, ns],
                            we_at(k)[:, cs],
                            start=(k == 0),
                            stop=(False if fuse_y else k == KT - 1),
                        )
                    if fuse_y:
                        # fuse the y broadcast-add into the accumulation
                        nc.tensor.matmul(
                            mps[ch], sel[:, b * P : (b + 1) * P], ysb[:BPC, cs],
                            start=False, stop=True,
                        )
                if mt < NEARLY:
                    # y not ready: drain to f32, add y in the hoisted epilogue
                    o = outpool.tile([P, 2, FD], F32, tag=f"o{mt}")
                    nc.scalar.copy(o[:, 0, :], mps[0])
                    nc.vector.tensor_copy(o[:, 1, :], mps[1])
                    o32.append(o)
                else:
                    # PSUM holds the final logits: drain fp16 on both engines
                    o = ostpool.tile([P, 2, FD], F16, tag="ost", name=f"ost{mt}")
                    nc.scalar.copy(o[:, 0, :], mps[0])
                    nc.vector.tensor_copy(o[:, 1, :], mps[1])
                    store(mt, o)

    nc.compile()
    _CACHE["nc"] = nc
    return nc


def _prep(inputs):
    """Host-side cast to fp16 + k-chunk layout transforms."""
    x = np.asarray(inputs["encoded_utterance"], np.float32)
    ee = np.asarray(inputs["element_embeddings"], np.float32)
    w = np.asarray(inputs["weight_matrix"], np.float32)
    wp = np.asarray(inputs["W_proj"], np.float32)
    bp = np.asarray(inputs["b_proj"], np.float32).reshape(1, D)

    # eet[p, b, k, n] = ee[b, n, k*128+p]  (partition-major)
    eet = np.ascontiguousarray(
        ee.reshape(B, N, KT, P).transpose(3, 0, 2, 1)
    ).astype(np.float16)

    # we/wu/wp[p, k, c] = W[k*128+p, c]
    def kchunk(m):
        return np.ascontiguousarray(
            m.reshape(KT, P, C).transpose(1, 0, 2)
        ).astype(np.float16)

    we_h = kchunk(w[D:])
    wu_h = kchunk(w[:D])
    wp_h = kchunk(wp)
    bp_h = bp.astype(np.float16)
    # xt[p, k, b] = x[b, k*128+p], per-core slice of b
    xt_full = np.ascontiguousarray(
        x.reshape(B, KT, P).transpose(2, 1, 0)
    ).astype(np.float16)
    sel_h = np.kron(np.eye(BPC), np.ones((1, P))).astype(np.float16)
    return eet, we_h, wu_h, wp_h, bp_h, xt_full, sel_h


def run(inputs, trace=False, **kwargs):
    nc = _build()
    eet, we_h, wu_h, wp_h, bp_h, xt_full, sel_h = _prep(inputs)

    in_maps = []
    for i in range(NCORES):
        bs = slice(i * BPC, (i + 1) * BPC)
        in_maps.append(
            {
                "eet": np.ascontiguousarray(eet[:, bs]),
                "we": we_h,
                "wu": wu_h,
                "wp": wp_h,
                "xt": np.ascontiguousarray(xt_full[:, :, bs]),
                "bp": bp_h,
                "sel": sel_h,
            }
        )

    res = run_bass_kernel_spmd(
        nc, in_maps, core_ids=list(range(NCORES)), trace=trace, **kwargs
    )
    full = np.concatenate([r["logits"] for r in res.results], axis=0)
    return full.astype(np.float32), res


def kernel(**inputs) -> np.ndarray:
    return run(inputs, trace=False)[0]
